# revision 34
# baseline (speedup 1.0000x reference)
"""Trainium2 Bass kernel for nn_ListenerModelBertAttCtxHist — ragged v5.

Data-parallel over the batch dim (64 -> 8 cores x 8 slots) with ragged
sequence packing (only unmasked positions shipped/computed; slot widths
baked per compiled program).

v5 (vs the 168us v2 baseline):
- Stage 1 runs e2h for all slots with the 16 W_ctx chunk-matmul groups
  interleaved, so the 12.6MB W_ctx stream and the e2h compute share the
  timeline. W_ctx stays replicated (hardware collectives cost ~75us fixed
  here - measured - so a ReduceScatter version loses).
- The ctx contribution to mm is produced ROW-major ([8, 512] via swapped
  matmul operands, b_mm folded in via a ones-row init matmul) and then
  injected into each slot's mm PSUM as a K=1 broadcast matmul. This
  deletes the entire fixup pass, the separate mmA copies, and the cbias
  transposes: mm+a1+a2 fuse into one pipelined stage 2 where the relu
  happens once, straight out of PSUM.
- Attention combine: one 3D multiply + one 3D reduce on vector (gpsimd
  bulk elementwise measured ~10x slower than DVE - don't use it).
- Startup: identities/ones via DMA; lhsT weights packed mt-major so the
  first matmul waits on only ~0.6MB.
- Sqrt (different scalar act table than Relu/Tanh/Exp) moved to the very
  end: exactly one table swap.
"""

import numpy as np
import ml_dtypes

import concourse.bacc as bacc
import concourse.mybir as mybir
import concourse.tile as tile
from concourse.bass_utils import run_bass_kernel_spmd

F32 = mybir.dt.float32
BF16 = mybir.dt.bfloat16

B, S, EMB, HID, IMG, ATT, K6, HL = 64, 512, 768, 512, 2048, 512, 6, 20
NCORES = 8
BL = B // NCORES
BK = BL * K6

BF = ml_dtypes.bfloat16
_NC_CACHE = {}


def _build_nc(widths):
    W0 = widths[0]
    cum = [0]
    for w in widths:
        cum.append(cum[-1] + w)
    SW = cum[-1]

    nc = bacc.Bacc("TRN2", target_bir_lowering=False, debug=False,
                   num_devices=NCORES)
    AF = mybir.ActivationFunctionType
    OP = mybir.AluOpType

    d_xt = nc.dram_tensor("xt", [128, K6 * SW], BF16, kind="ExternalInput")
    d_we2h = nc.dram_tensor("we2h", [128, K6 * HID], BF16, kind="ExternalInput")
    d_wmm = nc.dram_tensor("wmm", [128, 8 * HID], BF16, kind="ExternalInput")
    d_wa1 = nc.dram_tensor("wa1", [128, 4 * ATT], BF16, kind="ExternalInput")
    d_wa2 = nc.dram_tensor("wa2", [128, 4], BF16, kind="ExternalInput")
    d_whist = nc.dram_tensor("whist", [128, K6 * HID], BF16,
                             kind="ExternalInput")
    d_wsep = nc.dram_tensor("wsep", [128, 16 * HID], BF16,
                            kind="ExternalInput")
    d_wctx = nc.dram_tensor("wctx", [128, 96 * HID], BF16,
                            kind="ExternalInput")
    d_vct = nc.dram_tensor("vct", [128, 96 * BL], BF16, kind="ExternalInput")
    d_sit = nc.dram_tensor("sit", [128, 16 * BK], BF16, kind="ExternalInput")
    d_ph = nc.dram_tensor("ph", [2 * BK, HL * (EMB // 2)], BF16,
                          kind="ExternalInput")
    d_w96 = nc.dram_tensor("w96", [2 * BK, HL], F32, kind="ExternalInput")
    d_bias = nc.dram_tensor("bias", [128, 12], F32, kind="ExternalInput")
    d_rows = nc.dram_tensor("rows", [1, 4 * HID], BF16, kind="ExternalInput")
    d_mterm = nc.dram_tensor("mterm", [1, SW], BF16, kind="ExternalInput")
    d_g48 = nc.dram_tensor("g48", [1, BK], BF16, kind="ExternalInput")
    d_a48 = nc.dram_tensor("a48", [BL, BK], BF16, kind="ExternalInput")
    d_ones = nc.dram_tensor("ones", [1, 512], BF16, kind="ExternalInput")
    d_idf = nc.dram_tensor("idf", [128, 128], F32, kind="ExternalInput")
    d_idb = nc.dram_tensor("idb", [128, 128], BF16, kind="ExternalInput")
    d_out = nc.dram_tensor("out", [BK, 1], F32, kind="ExternalOutput")

    with tile.TileContext(nc) as tc:
        with (
            tc.tile_pool(name="const", bufs=1) as cw,
            tc.tile_pool(name="repp", bufs=8) as repp,
            tc.tile_pool(name="mmp", bufs=6) as mmp,
            tc.tile_pool(name="aTp", bufs=3) as aTp,
            tc.tile_pool(name="wbcp", bufs=2) as wbcp,
            tc.tile_pool(name="scrp", bufs=2) as scrp,
            tc.tile_pool(name="bp", bufs=2) as bp,
            tc.tile_pool(name="wctxp", bufs=6) as wctxp,
            tc.tile_pool(name="pbig", bufs=5, space="PSUM") as pbig,
            tc.tile_pool(name="pacc", bufs=1, space="PSUM") as pacc,
            tc.tile_pool(name="psmall", bufs=2, space="PSUM") as psmall,
        ):
            # ======== scalar HWDGE queue: stage-1 critical path ===========
            xt = cw.tile([128, K6 * SW], BF16)

            def xt_view(b):
                return xt[:, K6 * cum[b]:K6 * cum[b + 1]].rearrange(
                    "p (a n) -> p a n", a=K6)

            def xt_dma(eng, b):
                eng.dma_start(
                    xt[:, K6 * cum[b]:K6 * cum[b + 1]],
                    d_xt.ap()[:, K6 * cum[b]:K6 * cum[b + 1]])

            # scalar engine carries NO DMAs: descriptor-ring backpressure
            # on its sequencer would stall the PSUM-freeing relus.
            we2h = cw.tile([128, 4 * K6 * 128], BF16)
            nc.gpsimd.dma_start(we2h[:, :K6 * 128],
                                d_we2h.ap()[:, :K6 * 128])
            nc.gpsimd.dma_start(we2h[:, K6 * 128:], d_we2h.ap()[:, K6 * 128:])

            # ======== sync queue: descriptor generation costs ~0.6-1.1us
            # per dma_start on the sequencer, so the PE-critical xt0 goes
            # absolutely first; late-needed consts are emitted mid-loop. ==
            xt_dma(nc.sync, 0)
            biasp = cw.tile([128, 12], F32)
            nc.sync.dma_start(biasp[:], d_bias.ap())
            rows = cw.tile([1, 4 * HID], BF16)
            nc.sync.dma_start(rows[:], d_rows.ap())
            onesb = cw.tile([1, 512], BF16)
            nc.sync.dma_start(onesb[:], d_ones.ap())
            ones1 = onesb[:, 0:128]
            mterm = cw.tile([1, SW], BF16)
            g48 = cw.tile([1, BK], BF16)
            a48 = cw.tile([BL, BK], BF16)
            identf = cw.tile([128, 128], F32)
            identb = cw.tile([128, 128], BF16)

            # ======== gpsimd queue ========================================
            vct = cw.tile([128, 96 * BL], BF16)
            nc.gpsimd.dma_start(vct[:], d_vct.ap())
            phb = cw.tile([2 * BK, HL * (EMB // 2)], BF16)
            nc.gpsimd.dma_start(phb[:], d_ph.ap())
            w96b = cw.tile([2 * BK, HL], F32)
            nc.gpsimd.dma_start(w96b[:], d_w96.ap())
            wmm = cw.tile([128, 8 * 512], BF16)
            nc.gpsimd.dma_start(wmm[:], d_wmm.ap())
            wa1 = cw.tile([128, 4 * 512], BF16)
            nc.gpsimd.dma_start(wa1[:], d_wa1.ap())
            wa2 = cw.tile([128, 4], BF16)
            nc.gpsimd.dma_start(wa2[:], d_wa2.ap())
            whist = cw.tile([128, K6 * HID], BF16)
            nc.gpsimd.dma_start(whist[:], d_whist.ap())
            sit = cw.tile([128, 16 * BK], BF16)
            nc.gpsimd.dma_start(sit[:], d_sit.ap())
            wsep = cw.tile([128, 16 * HID], BF16)
            nc.gpsimd.dma_start(wsep[:], d_wsep.ap())

            # ======== helper emitters ======================================
            havg = cw.tile([2 * BK, EMB // 2], F32)
            havgT = cw.tile([128, K6, BK], BF16)
            hproj = cw.tile([BK, HID], F32)
            sep = cw.tile([BK, HID], F32)
            ssq = cw.tile([BK, 1], F32)
            rnorm = cw.tile([BK, 1], F32)

            def emit_hist_mult():
                w_bc = w96b[:].unsqueeze(2).broadcast_to(
                    [2 * BK, HL, EMB // 2])
                ph3 = phb[:].rearrange("p (l e) -> p l e", l=HL)
                nc.vector.tensor_tensor(ph3, ph3, w_bc, op=OP.mult)

            def emit_hist_reduce():
                half = EMB // 4
                for h in range(2):
                    hs = slice(h * half, (h + 1) * half)
                    nc.vector.tensor_reduce(
                        havg[:, hs],
                        phb[:].rearrange("p (l e) -> p e l", l=HL)[:, hs],
                        axis=mybir.AxisListType.X, op=OP.add)

            def emit_havgT():
                for j in range(3):
                    pt96 = psmall.tile([128, 2 * BK], F32, tag="small")
                    nc.tensor.transpose(pt96[:], havg[:, j * 128:(j + 1) * 128],
                                        identf[:2 * BK, :2 * BK])
                    for c in range(2):
                        nc.vector.tensor_copy(havgT[:, c * 3 + j, :],
                                              pt96[:, c * BK:(c + 1) * BK])

            def emit_norm():
                scr48 = cw.tile([BK, HID], F32)
                nc.scalar.activation(scr48[:], sep[:], AF.Square,
                                     accum_out=ssq[:])
                snorm = cw.tile([BK, 1], F32)
                nc.scalar.activation(snorm[:], ssq[:], AF.Sqrt)
                snormc = cw.tile([BK, 1], F32)
                nc.vector.tensor_scalar_max(snormc[:], snorm[:], 1e-12)
                nc.vector.reciprocal(rnorm[:], snormc[:])

            def emit_sep_hist_pe():
                php = pbig.tile([BK, HID], F32, tag="big")
                nc.tensor.matmul(php[:], g48[:], rows[:, 2 * HID:3 * HID],
                                 start=True, stop=False)
                for et in range(K6):
                    nc.tensor.matmul(php[:], havgT[:, et, :],
                                     whist[:, et * HID:(et + 1) * HID],
                                     start=False, stop=(et == K6 - 1))
                nc.scalar.activation(hproj[:], php[:], AF.Relu)
                psep = pbig.tile([BK, HID], F32, tag="big")
                nc.tensor.matmul(psep[:], onesb[:, :BK], rows[:, HID:2 * HID],
                                 start=True, stop=False)
                for kt in range(16):
                    nc.tensor.matmul(psep[:], sit[:, kt * BK:(kt + 1) * BK],
                                     wsep[:, kt * HID:(kt + 1) * HID],
                                     start=False, stop=(kt == 15))
                nc.vector.tensor_tensor(sep[:], psep[:], hproj[:], op=OP.add)
                nc.vector.tensor_scalar_max(sep[:], sep[:], 0.0)

            # ======== stage 1: e2h for all slots + W_ctx stream ===========
            pctx = pacc.tile([BL, HID], F32, tag="acc")
            nc.tensor.matmul(pctx[:], ones1[:, :BL], rows[:, 0:HID],
                             start=True, stop=False)

            def emit_ctx_chunk(ch, eng):
                wct = wctxp.tile([128, 6 * HID], BF16, tag="wc")
                eng.dma_start(wct[:], d_wctx.ap()[:, 6 * ch * HID:
                                                  (6 * ch + 6) * HID])
                for g in range(6):
                    gg = 6 * ch + g
                    nc.tensor.matmul(pctx[:], vct[:, gg * BL:(gg + 1) * BL],
                                     wct[:, g * HID:(g + 1) * HID],
                                     start=False, stop=(gg == 95))

            repsTs = {}
            for b in range(BL):
                W = widths[b]
                xv = xt_view(b)
                repsT = repp.tile([128, 4, W0], BF16, tag="repsT",
                                  name=f"repsT{b}")
                repsTs[b] = repsT
                for mt in range(4):
                    pe = pbig.tile([128, W0], F32, tag="big")
                    for kt in range(K6):
                        wsl = slice((mt * K6 + kt) * 128,
                                    (mt * K6 + kt) * 128 + 128)
                        nc.tensor.matmul(pe[:, :W], we2h[:, wsl],
                                         xv[:, kt, :],
                                         start=(kt == 0), stop=(kt == K6 - 1))
                    nc.scalar.activation(repsT[:, mt, :W], pe[:, :W],
                                         AF.Relu, bias=biasp[:, mt:mt + 1])
                if b + 1 < BL:
                    xt_dma(nc.sync, b + 1)
                emit_ctx_chunk(2 * b, nc.sync)
                emit_ctx_chunk(2 * b + 1, nc.sync)
                if b == 1:
                    nc.sync.dma_start(identf[:], d_idf.ap())
                    nc.sync.dma_start(identb[:], d_idb.ap())
                if b == 2:
                    nc.sync.dma_start(mterm[:], d_mterm.ap())
                if b == 3:
                    nc.sync.dma_start(g48[:], d_g48.ap())
                    nc.sync.dma_start(a48[:], d_a48.ap())
                if b == 2:
                    emit_hist_mult()
                if b == 3:
                    emit_hist_reduce()
                if b == 4:
                    emit_havgT()

            # ======== ctx post: relu, transpose, row-major mm bias ========
            ctxh = cw.tile([BL, HID], BF16)
            nc.scalar.activation(ctxh[:], pctx[:], AF.Relu)
            ctxT = cw.tile([128, 4, BL], BF16)
            for j in range(4):
                pt = psmall.tile([128, BL], BF16, tag="small")
                nc.tensor.transpose(pt[:], ctxh[:, j * 128:(j + 1) * 128],
                                    identb[:BL, :BL])
                nc.vector.tensor_copy(ctxT[:, j, :], pt[:])
            # cbiasT[:, mt, j] = b_mm + Wmm2^T ctx_j  (per-partition column
            # form: rides the mm relu as a free activation bias)
            cbiasT = cw.tile([128, 4, BL], F32)
            for mt in range(4):
                pcb = psmall.tile([128, BL], F32, tag="small")
                for kt in range(4):
                    wsl = slice((mt * 8 + 4 + kt) * 128,
                                (mt * 8 + 4 + kt) * 128 + 128)
                    nc.tensor.matmul(pcb[:], wmm[:, wsl], ctxT[:, kt, :],
                                     start=(kt == 0), stop=(kt == 3))
                nc.vector.tensor_scalar(cbiasT[:, mt, :], pcb[:],
                                        biasp[:, 4 + mt:5 + mt], None,
                                        op0=OP.add)

            # ======== stage 2: mm (+ctx bias in PSUM) -> a1 -> scores =====
            attT = cw.tile([128, 4 * BL], F32)
            mmTs = {}
            wbcs = {}
            attws = {}
            pscs = {}

            def emit_mm(j):
                Wj = widths[j]
                mmT = mmp.tile([128, 4, W0], BF16, tag="mmT", name=f"mmT{j}")
                mmTs[j] = mmT
                for mt in range(4):
                    pm = pbig.tile([128, W0], F32, tag="big")
                    for kt in range(4):
                        wsl = slice((mt * 8 + kt) * 128,
                                    (mt * 8 + kt) * 128 + 128)
                        nc.tensor.matmul(pm[:, :Wj], wmm[:, wsl],
                                         repsTs[j][:, kt, :Wj],
                                         start=(kt == 0), stop=(kt == 3))
                    nc.scalar.activation(mmT[:, mt, :Wj], pm[:, :Wj], AF.Relu,
                                         bias=cbiasT[:, mt, j:j + 1])

            def emit_attend(j, split=False):
                Wj = widths[j]
                scrb = scrp.tile([128, 4, W0], BF16, tag="scrb")
                groups = ((0, 2), (2, 4)) if split else ((0, 4),)
                for lo, hi in groups:
                    nc.vector.tensor_tensor(
                        scrb[:, lo:hi, :Wj], mmTs[j][:, lo:hi, :Wj],
                        wbcs[j][:, :Wj].unsqueeze(1).broadcast_to(
                            [128, hi - lo, Wj]),
                        op=OP.mult)
                    nc.vector.tensor_reduce(
                        attT[:, 4 * j + lo:4 * j + hi],
                        scrb[:, lo:hi, :Wj],
                        axis=mybir.AxisListType.X, op=OP.add)

            def emit_pwb(j):
                Wj = widths[j]
                wbc = wbcp.tile([128, W0], BF16, tag="wbc")
                wbcs[j] = wbc
                pwb = pbig.tile([128, W0], F32, tag="big")
                nc.tensor.matmul(pwb[:, :Wj], ones1, attws[j][:, :Wj],
                                 start=True, stop=True)
                nc.scalar.copy(wbc[:, :Wj], pwb[:, :Wj])

            def emit_soft(j):
                Wj = widths[j]
                sc = bp.tile([1, W0], F32, tag="sc")
                nc.vector.tensor_tensor(sc[:, :Wj], pscs[j][:, :Wj],
                                        mterm[:, cum[j]:cum[j] + Wj],
                                        op=OP.add)
                esc = bp.tile([1, W0], F32, tag="esc")
                zsum = bp.tile([1, 1], F32, tag="zsum")
                nc.scalar.activation(esc[:, :Wj], sc[:, :Wj], AF.Exp,
                                     accum_out=zsum[:])
                rz = bp.tile([1, 1], F32, tag="rz")
                nc.vector.reciprocal(rz[:], zsum[:])
                attw = bp.tile([1, W0], BF16, tag="attw")
                attws[j] = attw
                nc.vector.tensor_scalar_mul(attw[:, :Wj], esc[:, :Wj], rz[:])

            emit_mm(0)
            emit_mm(1)
            for b in range(BL):
                W = widths[b]
                mmT = mmTs[b]
                aT = aTp.tile([128, 4, W0], BF16, tag="aT")
                for mt in range(4):
                    pa = pbig.tile([128, W0], F32, tag="big")
                    for kt in range(4):
                        wsl = slice((mt * 4 + kt) * 128,
                                    (mt * 4 + kt) * 128 + 128)
                        nc.tensor.matmul(pa[:, :W], wa1[:, wsl],
                                         mmT[:, kt, :W],
                                         start=(kt == 0), stop=(kt == 3))
                    nc.scalar.activation(aT[:, mt, :W], pa[:, :W], AF.Tanh,
                                         bias=biasp[:, 8 + mt:9 + mt])
                    if mt == 0 and b >= 1:
                        emit_soft(b - 1)
                    if mt == 2 and b >= 1:
                        emit_pwb(b - 1)
                psc = psmall.tile([1, W0], F32, tag="small")
                pscs[b] = psc
                for kt in range(4):
                    nc.tensor.matmul(psc[:, :W], wa2[:, kt:kt + 1],
                                     aT[:, kt, :W],
                                     start=(kt == 0), stop=(kt == 3))
                if b == 2:
                    emit_sep_hist_pe()
                if b == 3:
                    emit_norm()
                if b + 2 < BL:
                    emit_mm(b + 2)
                if b >= 2:
                    emit_attend(b - 2)
            emit_attend(BL - 2, split=True)
            emit_soft(BL - 1)
            emit_pwb(BL - 1)
            emit_attend(BL - 1, split=True)

            # ======== finale ==============================================
            attended = cw.tile([BL, HID], BF16)
            attTv = attT[:].rearrange("p (b m) -> p m b", m=4)
            for mt in range(4):
                pt8 = psmall.tile([BL, 128], F32, tag="small")
                nc.tensor.transpose(pt8[:], attTv[:, mt, :], identf[:, :])
                nc.vector.tensor_copy(attended[:, mt * 128:(mt + 1) * 128],
                                      pt8[:])
            pa48 = pacc.tile([BK, HID], F32, tag="acc")
            nc.tensor.matmul(pa48[:], a48[:], attended[:],
                             start=True, stop=True)
            scr48b = cw.tile([BK, HID], F32)
            dotraw = cw.tile([BK, 1], F32)
            nc.vector.tensor_tensor(scr48b[:], sep[:], pa48[:], op=OP.mult)
            nc.vector.tensor_reduce(dotraw[:], scr48b[:],
                                    axis=mybir.AxisListType.X, op=OP.add)
            dotf = cw.tile([BK, 1], F32)
            nc.vector.tensor_scalar_mul(dotf[:], dotraw[:], rnorm[:])
            nc.sync.dma_start(d_out.ap(), dotf[:])

    nc.compile()
    return nc


def _get_nc(widths):
    key = tuple(widths)
    if key not in _NC_CACHE:
        _NC_CACHE[key] = _build_nc(key)
    return _NC_CACHE[key]


def _t128(w, a):
    h = w.shape[1]
    return np.ascontiguousarray(
        w.astype(BF).reshape(a, 128, h).transpose(1, 0, 2)).reshape(128, a * h)


def _tmt(w, kt):
    # mt-major lhsT pack: out[p, mt, kt, j] = w[kt*128 + p, mt*128 + j]
    return np.ascontiguousarray(
        w.astype(BF).reshape(kt, 128, 4, 128).transpose(1, 2, 0, 3)
    ).reshape(128, kt * 512)


def _plan(masks):
    nk = (~masks.reshape(B, S)).sum(1)
    perm = np.argsort(-nk, kind="stable")
    widths = []
    for s in range(BL):
        w = int(nk[perm[s * NCORES]])
        w = min(max((w + 15) // 16 * 16, 16), S)
        widths.append(w)
    return perm, tuple(widths), nk


def _make_in_maps(inputs, perm, widths):
    reps = np.asarray(inputs["representations"], dtype=np.float32)
    si = np.asarray(inputs["separate_images"], dtype=np.float32)
    vc = np.asarray(inputs["visual_context"], dtype=np.float32)
    ph = np.asarray(inputs["prev_hist"], dtype=np.float32)
    cnts = np.asarray(inputs["hist_counts"]).astype(np.float32)
    msks = np.asarray(inputs["masks"]).astype(bool).reshape(B, S)
    SW = sum(widths)
    cum = np.concatenate([[0], np.cumsum(widths)]).astype(int)

    bias = np.concatenate([
        np.asarray(inputs["b_e2h"], np.float32).reshape(4, 128),
        np.asarray(inputs["b_mm"], np.float32).reshape(4, 128),
        np.asarray(inputs["b_a1"], np.float32).reshape(4, 128)], 0).T
    rows = np.concatenate([
        np.asarray(inputs["b_ctx"], np.float32),
        np.asarray(inputs["b_sep"], np.float32),
        np.asarray(inputs["b_hist"], np.float32),
        np.asarray(inputs["b_mm"], np.float32)]).reshape(1, 4 * HID)
    a48 = (np.arange(BK)[None, :] // K6 ==
           np.arange(BL)[:, None]).astype(np.float32)
    ident = np.eye(128, dtype=np.float32)

    shared = {
        "we2h": _tmt(np.asarray(inputs["W_e2h"], np.float32), K6),
        "wmm": _tmt(np.asarray(inputs["W_mm"], np.float32), 8),
        "wa1": _tmt(np.asarray(inputs["W_a1"], np.float32), 4),
        "wa2": np.ascontiguousarray(
            np.asarray(inputs["W_a2"], np.float32).reshape(4, 128).T
        ).astype(BF),
        "whist": _t128(np.asarray(inputs["W_hist"], np.float32), K6),
        "wsep": _t128(np.asarray(inputs["W_sep"], np.float32), 16),
        "wctx": _t128(np.asarray(inputs["W_ctx"], np.float32), 96),
        "bias": np.ascontiguousarray(bias),
        "rows": rows.astype(BF),
        "a48": a48.astype(BF),
        "ones": np.ones((1, 512), np.float32).astype(BF),
        "idf": ident,
        "idb": ident.astype(BF),
    }

    in_maps = []
    for c in range(NCORES):
        gb = [int(perm[s * NCORES + c]) for s in range(BL)]  # slot -> batch
        m = dict(shared)
        xtc = np.zeros((128, K6 * SW), dtype=BF)
        mt = np.zeros((1, SW), dtype=np.float32)
        for s, g in enumerate(gb):
            W = widths[s]
            keep = np.flatnonzero(~msks[g])
            k = min(len(keep), W)
            arr = np.zeros((W, EMB), dtype=np.float32)
            arr[:k] = reps[g, keep[:k]]
            blk = arr.astype(BF).reshape(W, K6, 128).transpose(2, 1, 0)
            xtc[:, K6 * cum[s]:K6 * cum[s + 1]] = blk.reshape(128, K6 * W)
            mt[0, cum[s]:cum[s] + W] = np.where(np.arange(W) < k, 0.0, -1e30)
        m["xt"] = np.ascontiguousarray(xtc)
        m["mterm"] = mt.astype(BF)
        m["sit"] = _t128(si[gb].reshape(BK, IMG).T.copy(), 16)
        m["vct"] = np.ascontiguousarray(
            vc[gb].T.astype(BF).reshape(96, 128, BL)
            .transpose(1, 0, 2)).reshape(128, 96 * BL)
        m["ph"] = np.ascontiguousarray(
            ph[gb].astype(BF).reshape(BK, HL, 2, EMB // 2)
            .transpose(2, 0, 1, 3)).reshape(2 * BK, HL * (EMB // 2))
        cnt = cnts[gb].reshape(BK)
        valid = (np.arange(HL)[None, :] < cnt[:, None]).astype(np.float32)
        w48 = valid / np.maximum(cnt, 1.0)[:, None]
        m["w96"] = np.ascontiguousarray(np.tile(w48, (2, 1)))
        m["g48"] = (cnt > 0).astype(np.float32).reshape(1, BK).astype(BF)
        in_maps.append(m)
    return in_maps


def run(inputs, trace=False, trace_kwargs={}, run_kwargs={}):
    masks = np.asarray(inputs["masks"]).astype(bool)
    perm, widths, nk = _plan(masks)
    nc = _get_nc(widths)
    in_maps = _make_in_maps(inputs, perm, widths)
    res = run_bass_kernel_spmd(nc, in_maps, core_ids=list(range(NCORES)),
                               trace=trace, trace_kwargs=trace_kwargs,
                               **run_kwargs)
    out = np.zeros((B, K6, 1), dtype=np.float32)
    for c in range(NCORES):
        oc = res.results[c]["out"].reshape(BL, K6)
        for s in range(BL):
            out[perm[s * NCORES + c], :, 0] = oc[s]
    return out, res


def kernel(**inputs):
    out, _ = run(inputs, trace=False)
    return out


# revision 35
# speedup vs baseline: 1.0118x; 1.0118x over previous
"""Trainium2 Bass kernel for nn_ListenerModelBertAttCtxHist — ragged v7.

Data-parallel over the batch dim (64 -> 8 cores x 8 slots) with ragged
sequence packing (only unmasked positions shipped/computed; slot widths
baked per compiled program, cached per width tuple).

Design (~157us, vs 187.5us baseline):
- Stage 1: e2h for all 8 slots interleaved with the 16 W_ctx chunk-matmul
  groups; the 12.6MB replicated W_ctx stream paces this stage (hardware
  collectives cost ~75us fixed in this environment - measured - so a
  ReduceScatter-sharded W_ctx loses).
- DMA lessons baked in: per dma_start the issuing sequencer burns
  ~0.6-1.1us generating descriptors, so the PE-critical xt0 descriptor is
  first on sync, late-needed consts are deferred mid-loop, the scalar
  engine (whose compute frees PSUM) carries no DMAs at all, and xt slots
  are interleaved with wctx chunks on one queue in exact PE consumption
  order. All bulk transfers are flat 2D copies.
- ctx bias: pcb = Wmm2^T ctx (+b_mm) is kept in per-partition column form
  and folded into the mm relu as a free scalar-activation bias.
- Stage 2: mm -> a1 -> a2 software-pipelined per slot (mm emitted 2 slots
  ahead); softmax/broadcast/attention-combine slotted between matmul
  groups. Attention combine is a 3D multiply + 3D reduce on vector
  (gpsimd bulk elementwise measured ~10x slower than DVE).
- sep/hist (PE matmuls with host-prepared averaging weights) and the L2
  norm run inside stage 2; Sqrt sits in a different scalar act table than
  Relu/Tanh/Exp so it is placed to swap tables exactly once.
"""

import numpy as np
import ml_dtypes

import concourse.bacc as bacc
import concourse.mybir as mybir
import concourse.tile as tile
from concourse.bass_utils import run_bass_kernel_spmd

F32 = mybir.dt.float32
BF16 = mybir.dt.bfloat16

B, S, EMB, HID, IMG, ATT, K6, HL = 64, 512, 768, 512, 2048, 512, 6, 20
NCORES = 8
BL = B // NCORES
BK = BL * K6

BF = ml_dtypes.bfloat16
_NC_CACHE = {}


def _build_nc(widths):
    W0 = widths[0]
    cum = [0]
    for w in widths:
        cum.append(cum[-1] + w)
    SW = cum[-1]

    nc = bacc.Bacc("TRN2", target_bir_lowering=False, debug=False,
                   num_devices=NCORES)
    AF = mybir.ActivationFunctionType
    OP = mybir.AluOpType

    d_xt = nc.dram_tensor("xt", [128, K6 * SW], BF16, kind="ExternalInput")
    d_we2h = nc.dram_tensor("we2h", [128, K6 * HID], BF16, kind="ExternalInput")
    d_wmm = nc.dram_tensor("wmm", [128, 8 * HID], BF16, kind="ExternalInput")
    d_wa1 = nc.dram_tensor("wa1", [128, 4 * ATT], BF16, kind="ExternalInput")
    d_wa2 = nc.dram_tensor("wa2", [128, 4], BF16, kind="ExternalInput")
    d_whist = nc.dram_tensor("whist", [128, K6 * HID], BF16,
                             kind="ExternalInput")
    d_wsep = nc.dram_tensor("wsep", [128, 16 * HID], BF16,
                            kind="ExternalInput")
    d_wctx = nc.dram_tensor("wctx", [128, 96 * HID], BF16,
                            kind="ExternalInput")
    d_vct = nc.dram_tensor("vct", [128, 96 * BL], BF16, kind="ExternalInput")
    d_sit = nc.dram_tensor("sit", [128, 16 * BK], BF16, kind="ExternalInput")
    d_ph = nc.dram_tensor("ph", [2 * BK, HL * (EMB // 2)], BF16,
                          kind="ExternalInput")
    d_w96 = nc.dram_tensor("w96", [2 * BK, HL], F32, kind="ExternalInput")
    d_bias = nc.dram_tensor("bias", [128, 12], F32, kind="ExternalInput")
    d_rows = nc.dram_tensor("rows", [1, 4 * HID], BF16, kind="ExternalInput")
    d_mterm = nc.dram_tensor("mterm", [1, SW], BF16, kind="ExternalInput")
    d_g48 = nc.dram_tensor("g48", [1, BK], BF16, kind="ExternalInput")
    d_a48 = nc.dram_tensor("a48", [BL, BK], BF16, kind="ExternalInput")
    d_ones = nc.dram_tensor("ones", [1, 512], BF16, kind="ExternalInput")
    d_idf = nc.dram_tensor("idf", [128, 128], F32, kind="ExternalInput")
    d_idb = nc.dram_tensor("idb", [128, 128], BF16, kind="ExternalInput")
    d_out = nc.dram_tensor("out", [BK, 1], F32, kind="ExternalOutput")

    with tile.TileContext(nc) as tc:
        with (
            tc.tile_pool(name="const", bufs=1) as cw,
            tc.tile_pool(name="repp", bufs=8) as repp,
            tc.tile_pool(name="mmp", bufs=6) as mmp,
            tc.tile_pool(name="aTp", bufs=3) as aTp,
            tc.tile_pool(name="wbcp", bufs=2) as wbcp,
            tc.tile_pool(name="scrp", bufs=2) as scrp,
            tc.tile_pool(name="bp", bufs=2) as bp,
            tc.tile_pool(name="wctxp", bufs=6) as wctxp,
            tc.tile_pool(name="pbig", bufs=4, space="PSUM") as pbig,
            tc.tile_pool(name="pacc", bufs=2, space="PSUM") as pacc,
            tc.tile_pool(name="psmall", bufs=2, space="PSUM") as psmall,
        ):
            # ======== scalar HWDGE queue: stage-1 critical path ===========
            xt = cw.tile([128, K6 * SW], BF16)

            def xt_view(b):
                return xt[:, K6 * cum[b]:K6 * cum[b + 1]].rearrange(
                    "p (a n) -> p a n", a=K6)

            def xt_dma(eng, b):
                eng.dma_start(
                    xt[:, K6 * cum[b]:K6 * cum[b + 1]],
                    d_xt.ap()[:, K6 * cum[b]:K6 * cum[b + 1]])

            # scalar engine carries NO DMAs: descriptor-ring backpressure
            # on its sequencer would stall the PSUM-freeing relus.
            we2h = cw.tile([128, 4 * K6 * 128], BF16)
            nc.gpsimd.dma_start(we2h[:, :K6 * 128],
                                d_we2h.ap()[:, :K6 * 128])
            nc.gpsimd.dma_start(we2h[:, K6 * 128:], d_we2h.ap()[:, K6 * 128:])

            # ======== sync queue: descriptor generation costs ~0.6-1.1us
            # per dma_start on the sequencer, so the PE-critical xt0 goes
            # absolutely first; late-needed consts are emitted mid-loop. ==
            xt_dma(nc.sync, 0)
            biasp = cw.tile([128, 12], F32)
            nc.sync.dma_start(biasp[:], d_bias.ap())
            rows = cw.tile([1, 4 * HID], BF16)
            nc.sync.dma_start(rows[:], d_rows.ap())
            onesb = cw.tile([1, 512], BF16)
            nc.sync.dma_start(onesb[:], d_ones.ap())
            ones1 = onesb[:, 0:128]
            mterm = cw.tile([1, SW], BF16)
            g48 = cw.tile([1, BK], BF16)
            a48 = cw.tile([BL, BK], BF16)
            identf = cw.tile([128, 128], F32)
            identb = cw.tile([128, 128], BF16)

            # ======== gpsimd queue ========================================
            vct = cw.tile([128, 96 * BL], BF16)
            nc.gpsimd.dma_start(vct[:], d_vct.ap())
            phb = cw.tile([2 * BK, HL * (EMB // 2)], BF16)
            nc.gpsimd.dma_start(phb[:], d_ph.ap())
            w96b = cw.tile([2 * BK, HL], F32)
            nc.gpsimd.dma_start(w96b[:], d_w96.ap())
            wmm = cw.tile([128, 8 * 512], BF16)
            nc.gpsimd.dma_start(wmm[:], d_wmm.ap())
            wa1 = cw.tile([128, 4 * 512], BF16)
            nc.gpsimd.dma_start(wa1[:], d_wa1.ap())
            wa2 = cw.tile([128, 4], BF16)
            nc.gpsimd.dma_start(wa2[:], d_wa2.ap())
            whist = cw.tile([128, K6 * HID], BF16)
            nc.gpsimd.dma_start(whist[:], d_whist.ap())
            sit = cw.tile([128, 16 * BK], BF16)
            nc.gpsimd.dma_start(sit[:], d_sit.ap())
            wsep = cw.tile([128, 16 * HID], BF16)
            nc.gpsimd.dma_start(wsep[:], d_wsep.ap())

            # ======== helper emitters ======================================
            havg = cw.tile([2 * BK, EMB // 2], F32)
            havgT = cw.tile([128, K6, BK], BF16)
            hproj = cw.tile([BK, HID], F32)
            sep = cw.tile([BK, HID], F32)
            ssq = cw.tile([BK, 1], F32)
            rnorm = cw.tile([BK, 1], F32)

            def emit_hist_mult():
                w_bc = w96b[:].unsqueeze(2).broadcast_to(
                    [2 * BK, HL, EMB // 2])
                ph3 = phb[:].rearrange("p (l e) -> p l e", l=HL)
                nc.vector.tensor_tensor(ph3, ph3, w_bc, op=OP.mult)

            def emit_hist_reduce():
                half = EMB // 4
                for h in range(2):
                    hs = slice(h * half, (h + 1) * half)
                    nc.vector.tensor_reduce(
                        havg[:, hs],
                        phb[:].rearrange("p (l e) -> p e l", l=HL)[:, hs],
                        axis=mybir.AxisListType.X, op=OP.add)

            def emit_havgT():
                for j in range(3):
                    pt96 = psmall.tile([128, 2 * BK], F32, tag="small")
                    nc.tensor.transpose(pt96[:], havg[:, j * 128:(j + 1) * 128],
                                        identf[:2 * BK, :2 * BK])
                    for c in range(2):
                        nc.vector.tensor_copy(havgT[:, c * 3 + j, :],
                                              pt96[:, c * BK:(c + 1) * BK])

            def emit_norm():
                scr48 = cw.tile([BK, HID], F32)
                nc.scalar.activation(scr48[:], sep[:], AF.Square,
                                     accum_out=ssq[:])
                snorm = cw.tile([BK, 1], F32)
                nc.scalar.activation(snorm[:], ssq[:], AF.Sqrt)
                snormc = cw.tile([BK, 1], F32)
                nc.vector.tensor_scalar_max(snormc[:], snorm[:], 1e-12)
                nc.vector.reciprocal(rnorm[:], snormc[:])

            def emit_sep_hist_pe():
                php = pbig.tile([BK, HID], F32, tag="big")
                nc.tensor.matmul(php[:], g48[:], rows[:, 2 * HID:3 * HID],
                                 start=True, stop=False)
                for et in range(K6):
                    nc.tensor.matmul(php[:], havgT[:, et, :],
                                     whist[:, et * HID:(et + 1) * HID],
                                     start=False, stop=(et == K6 - 1))
                nc.scalar.activation(hproj[:], php[:], AF.Relu)
                psep = pbig.tile([BK, HID], F32, tag="big")
                nc.tensor.matmul(psep[:], onesb[:, :BK], rows[:, HID:2 * HID],
                                 start=True, stop=False)
                for kt in range(16):
                    nc.tensor.matmul(psep[:], sit[:, kt * BK:(kt + 1) * BK],
                                     wsep[:, kt * HID:(kt + 1) * HID],
                                     start=False, stop=(kt == 15))
                nc.vector.tensor_tensor(sep[:], psep[:], hproj[:], op=OP.add)
                nc.vector.tensor_scalar_max(sep[:], sep[:], 0.0)

            # ======== stage 1: e2h for all slots + W_ctx stream ===========
            pctx = pacc.tile([BL, HID], F32, tag="acc")
            nc.tensor.matmul(pctx[:], ones1[:, :BL], rows[:, 0:HID],
                             start=True, stop=False)

            def emit_ctx_chunk(ch, eng):
                wct = wctxp.tile([128, 6 * HID], BF16, tag="wc")
                eng.dma_start(wct[:], d_wctx.ap()[:, 6 * ch * HID:
                                                  (6 * ch + 6) * HID])
                for g in range(6):
                    gg = 6 * ch + g
                    nc.tensor.matmul(pctx[:], vct[:, gg * BL:(gg + 1) * BL],
                                     wct[:, g * HID:(g + 1) * HID],
                                     start=False, stop=(gg == 95))

            repsTs = {}
            for b in range(BL):
                W = widths[b]
                xv = xt_view(b)
                repsT = repp.tile([128, 4, W0], BF16, tag="repsT",
                                  name=f"repsT{b}")
                repsTs[b] = repsT
                for mt in range(4):
                    pe = pbig.tile([128, W0], F32, tag="big")
                    for kt in range(K6):
                        wsl = slice((mt * K6 + kt) * 128,
                                    (mt * K6 + kt) * 128 + 128)
                        nc.tensor.matmul(pe[:, :W], we2h[:, wsl],
                                         xv[:, kt, :],
                                         start=(kt == 0), stop=(kt == K6 - 1))
                    nc.scalar.activation(repsT[:, mt, :W], pe[:, :W],
                                         AF.Relu, bias=biasp[:, mt:mt + 1])
                if b + 1 < BL:
                    xt_dma(nc.sync, b + 1)
                emit_ctx_chunk(2 * b, nc.sync)
                emit_ctx_chunk(2 * b + 1, nc.sync)
                if b == 1:
                    nc.sync.dma_start(identf[:], d_idf.ap())
                    nc.sync.dma_start(identb[:], d_idb.ap())
                if b == 2:
                    nc.sync.dma_start(mterm[:], d_mterm.ap())
                if b == 3:
                    nc.sync.dma_start(g48[:], d_g48.ap())
                    nc.sync.dma_start(a48[:], d_a48.ap())
                if b == 2:
                    emit_hist_mult()
                if b == 3:
                    emit_hist_reduce()
                if b == 4:
                    emit_havgT()

            # ======== ctx post: relu, transpose, row-major mm bias ========
            ctxh = cw.tile([BL, HID], BF16)
            nc.scalar.activation(ctxh[:], pctx[:], AF.Relu)
            ctxT = cw.tile([128, 4, BL], BF16)
            for j in range(4):
                pt = psmall.tile([128, BL], BF16, tag="small")
                nc.tensor.transpose(pt[:], ctxh[:, j * 128:(j + 1) * 128],
                                    identb[:BL, :BL])
                nc.vector.tensor_copy(ctxT[:, j, :], pt[:])
            # cbiasT[:, mt, j] = b_mm + Wmm2^T ctx_j  (per-partition column
            # form: rides the mm relu as a free activation bias)
            cbiasT = cw.tile([128, 4, BL], F32)
            for mt in range(4):
                pcb = psmall.tile([128, BL], F32, tag="small")
                for kt in range(4):
                    wsl = slice((mt * 8 + 4 + kt) * 128,
                                (mt * 8 + 4 + kt) * 128 + 128)
                    nc.tensor.matmul(pcb[:], wmm[:, wsl], ctxT[:, kt, :],
                                     start=(kt == 0), stop=(kt == 3))
                nc.vector.tensor_scalar(cbiasT[:, mt, :], pcb[:],
                                        biasp[:, 4 + mt:5 + mt], None,
                                        op0=OP.add)

            # ======== stage 2: mm (+ctx bias in PSUM) -> a1 -> scores =====
            attT = cw.tile([128, 4 * BL], F32)
            mmTs = {}
            wbcs = {}
            attws = {}
            pscs = {}

            def emit_mm(j):
                Wj = widths[j]
                mmT = mmp.tile([128, 4, W0], BF16, tag="mmT", name=f"mmT{j}")
                mmTs[j] = mmT
                for mt in range(4):
                    pm = pbig.tile([128, W0], F32, tag="big")
                    for kt in range(4):
                        wsl = slice((mt * 8 + kt) * 128,
                                    (mt * 8 + kt) * 128 + 128)
                        nc.tensor.matmul(pm[:, :Wj], wmm[:, wsl],
                                         repsTs[j][:, kt, :Wj],
                                         start=(kt == 0), stop=(kt == 3))
                    nc.scalar.activation(mmT[:, mt, :Wj], pm[:, :Wj], AF.Relu,
                                         bias=cbiasT[:, mt, j:j + 1])

            def emit_attend(j, split=False):
                Wj = widths[j]
                scrb = scrp.tile([128, 4, W0], BF16, tag="scrb")
                groups = ((0, 2), (2, 4)) if split else ((0, 4),)
                for lo, hi in groups:
                    nc.vector.tensor_tensor(
                        scrb[:, lo:hi, :Wj], mmTs[j][:, lo:hi, :Wj],
                        wbcs[j][:, :Wj].unsqueeze(1).broadcast_to(
                            [128, hi - lo, Wj]),
                        op=OP.mult)
                    nc.vector.tensor_reduce(
                        attT[:, 4 * j + lo:4 * j + hi],
                        scrb[:, lo:hi, :Wj],
                        axis=mybir.AxisListType.X, op=OP.add)

            def emit_pwb(j):
                Wj = widths[j]
                wbc = wbcp.tile([128, W0], BF16, tag="wbc")
                wbcs[j] = wbc
                pwb = pbig.tile([128, W0], F32, tag="big")
                nc.tensor.matmul(pwb[:, :Wj], ones1, attws[j][:, :Wj],
                                 start=True, stop=True)
                nc.scalar.copy(wbc[:, :Wj], pwb[:, :Wj])

            def emit_soft(j):
                Wj = widths[j]
                sc = bp.tile([1, W0], F32, tag="sc")
                nc.vector.tensor_tensor(sc[:, :Wj], pscs[j][:, :Wj],
                                        mterm[:, cum[j]:cum[j] + Wj],
                                        op=OP.add)
                esc = bp.tile([1, W0], F32, tag="esc")
                zsum = bp.tile([1, 1], F32, tag="zsum")
                nc.scalar.activation(esc[:, :Wj], sc[:, :Wj], AF.Exp,
                                     accum_out=zsum[:])
                rz = bp.tile([1, 1], F32, tag="rz")
                nc.vector.reciprocal(rz[:], zsum[:])
                attw = bp.tile([1, W0], BF16, tag="attw")
                attws[j] = attw
                nc.vector.tensor_scalar_mul(attw[:, :Wj], esc[:, :Wj], rz[:])

            emit_mm(0)
            emit_mm(1)
            for b in range(BL):
                W = widths[b]
                mmT = mmTs[b]
                aT = aTp.tile([128, 4, W0], BF16, tag="aT")
                for mt in range(4):
                    pa = pbig.tile([128, W0], F32, tag="big")
                    for kt in range(4):
                        wsl = slice((mt * 4 + kt) * 128,
                                    (mt * 4 + kt) * 128 + 128)
                        nc.tensor.matmul(pa[:, :W], wa1[:, wsl],
                                         mmT[:, kt, :W],
                                         start=(kt == 0), stop=(kt == 3))
                    nc.scalar.activation(aT[:, mt, :W], pa[:, :W], AF.Tanh,
                                         bias=biasp[:, 8 + mt:9 + mt])
                    if mt == 0 and b >= 1:
                        emit_soft(b - 1)
                    if mt == 2 and b >= 1:
                        emit_pwb(b - 1)
                psc = psmall.tile([1, W0], F32, tag="small")
                pscs[b] = psc
                for kt in range(4):
                    nc.tensor.matmul(psc[:, :W], wa2[:, kt:kt + 1],
                                     aT[:, kt, :W],
                                     start=(kt == 0), stop=(kt == 3))
                if b == 2:
                    emit_sep_hist_pe()
                if b == 3:
                    emit_norm()
                if b + 2 < BL:
                    emit_mm(b + 2)
                if b >= 2:
                    emit_attend(b - 2)
            emit_attend(BL - 2, split=True)
            emit_soft(BL - 1)
            emit_pwb(BL - 1)
            emit_attend(BL - 1, split=True)

            # ======== finale ==============================================
            attended = cw.tile([BL, HID], BF16)
            attTv = attT[:].rearrange("p (b m) -> p m b", m=4)
            for mt in range(4):
                pt8 = psmall.tile([BL, 128], F32, tag="small")
                nc.tensor.transpose(pt8[:], attTv[:, mt, :], identf[:, :])
                nc.vector.tensor_copy(attended[:, mt * 128:(mt + 1) * 128],
                                      pt8[:])
            pa48 = pacc.tile([BK, HID], F32, tag="acc")
            nc.tensor.matmul(pa48[:], a48[:], attended[:],
                             start=True, stop=True)
            scr48b = cw.tile([BK, HID], F32)
            dotraw = cw.tile([BK, 1], F32)
            nc.vector.tensor_tensor(scr48b[:], sep[:], pa48[:], op=OP.mult)
            nc.vector.tensor_reduce(dotraw[:], scr48b[:],
                                    axis=mybir.AxisListType.X, op=OP.add)
            dotf = cw.tile([BK, 1], F32)
            nc.vector.tensor_scalar_mul(dotf[:], dotraw[:], rnorm[:])
            nc.sync.dma_start(d_out.ap(), dotf[:])

    nc.compile()
    return nc


def _get_nc(widths):
    key = tuple(widths)
    if key not in _NC_CACHE:
        _NC_CACHE[key] = _build_nc(key)
    return _NC_CACHE[key]


def _t128(w, a):
    h = w.shape[1]
    return np.ascontiguousarray(
        w.astype(BF).reshape(a, 128, h).transpose(1, 0, 2)).reshape(128, a * h)


def _tmt(w, kt):
    # mt-major lhsT pack: out[p, mt, kt, j] = w[kt*128 + p, mt*128 + j]
    return np.ascontiguousarray(
        w.astype(BF).reshape(kt, 128, 4, 128).transpose(1, 2, 0, 3)
    ).reshape(128, kt * 512)


def _plan(masks):
    nk = (~masks.reshape(B, S)).sum(1)
    perm = np.argsort(-nk, kind="stable")
    widths = []
    for s in range(BL):
        w = int(nk[perm[s * NCORES]])
        w = min(max((w + 15) // 16 * 16, 16), S)
        widths.append(w)
    return perm, tuple(widths), nk


def _make_in_maps(inputs, perm, widths):
    reps = np.asarray(inputs["representations"], dtype=np.float32)
    si = np.asarray(inputs["separate_images"], dtype=np.float32)
    vc = np.asarray(inputs["visual_context"], dtype=np.float32)
    ph = np.asarray(inputs["prev_hist"], dtype=np.float32)
    cnts = np.asarray(inputs["hist_counts"]).astype(np.float32)
    msks = np.asarray(inputs["masks"]).astype(bool).reshape(B, S)
    SW = sum(widths)
    cum = np.concatenate([[0], np.cumsum(widths)]).astype(int)

    bias = np.concatenate([
        np.asarray(inputs["b_e2h"], np.float32).reshape(4, 128),
        np.asarray(inputs["b_mm"], np.float32).reshape(4, 128),
        np.asarray(inputs["b_a1"], np.float32).reshape(4, 128)], 0).T
    rows = np.concatenate([
        np.asarray(inputs["b_ctx"], np.float32),
        np.asarray(inputs["b_sep"], np.float32),
        np.asarray(inputs["b_hist"], np.float32),
        np.asarray(inputs["b_mm"], np.float32)]).reshape(1, 4 * HID)
    a48 = (np.arange(BK)[None, :] // K6 ==
           np.arange(BL)[:, None]).astype(np.float32)
    ident = np.eye(128, dtype=np.float32)

    shared = {
        "we2h": _tmt(np.asarray(inputs["W_e2h"], np.float32), K6),
        "wmm": _tmt(np.asarray(inputs["W_mm"], np.float32), 8),
        "wa1": _tmt(np.asarray(inputs["W_a1"], np.float32), 4),
        "wa2": np.ascontiguousarray(
            np.asarray(inputs["W_a2"], np.float32).reshape(4, 128).T
        ).astype(BF),
        "whist": _t128(np.asarray(inputs["W_hist"], np.float32), K6),
        "wsep": _t128(np.asarray(inputs["W_sep"], np.float32), 16),
        "wctx": _t128(np.asarray(inputs["W_ctx"], np.float32), 96),
        "bias": np.ascontiguousarray(bias),
        "rows": rows.astype(BF),
        "a48": a48.astype(BF),
        "ones": np.ones((1, 512), np.float32).astype(BF),
        "idf": ident,
        "idb": ident.astype(BF),
    }

    in_maps = []
    for c in range(NCORES):
        gb = [int(perm[s * NCORES + c]) for s in range(BL)]  # slot -> batch
        m = dict(shared)
        xtc = np.zeros((128, K6 * SW), dtype=BF)
        mt = np.zeros((1, SW), dtype=np.float32)
        for s, g in enumerate(gb):
            W = widths[s]
            keep = np.flatnonzero(~msks[g])
            k = min(len(keep), W)
            arr = np.zeros((W, EMB), dtype=np.float32)
            arr[:k] = reps[g, keep[:k]]
            blk = arr.astype(BF).reshape(W, K6, 128).transpose(2, 1, 0)
            xtc[:, K6 * cum[s]:K6 * cum[s + 1]] = blk.reshape(128, K6 * W)
            mt[0, cum[s]:cum[s] + W] = np.where(np.arange(W) < k, 0.0, -1e30)
        m["xt"] = np.ascontiguousarray(xtc)
        m["mterm"] = mt.astype(BF)
        m["sit"] = _t128(si[gb].reshape(BK, IMG).T.copy(), 16)
        m["vct"] = np.ascontiguousarray(
            vc[gb].T.astype(BF).reshape(96, 128, BL)
            .transpose(1, 0, 2)).reshape(128, 96 * BL)
        m["ph"] = np.ascontiguousarray(
            ph[gb].astype(BF).reshape(BK, HL, 2, EMB // 2)
            .transpose(2, 0, 1, 3)).reshape(2 * BK, HL * (EMB // 2))
        cnt = cnts[gb].reshape(BK)
        valid = (np.arange(HL)[None, :] < cnt[:, None]).astype(np.float32)
        w48 = valid / np.maximum(cnt, 1.0)[:, None]
        m["w96"] = np.ascontiguousarray(np.tile(w48, (2, 1)))
        m["g48"] = (cnt > 0).astype(np.float32).reshape(1, BK).astype(BF)
        in_maps.append(m)
    return in_maps


def run(inputs, trace=False, trace_kwargs={}, run_kwargs={}):
    masks = np.asarray(inputs["masks"]).astype(bool)
    perm, widths, nk = _plan(masks)
    nc = _get_nc(widths)
    in_maps = _make_in_maps(inputs, perm, widths)
    res = run_bass_kernel_spmd(nc, in_maps, core_ids=list(range(NCORES)),
                               trace=trace, trace_kwargs=trace_kwargs,
                               **run_kwargs)
    out = np.zeros((B, K6, 1), dtype=np.float32)
    for c in range(NCORES):
        oc = res.results[c]["out"].reshape(BL, K6)
        for s in range(BL):
            out[perm[s * NCORES + c], :, 0] = oc[s]
    return out, res


def kernel(**inputs):
    out, _ = run(inputs, trace=False)
    return out
